# revision 11
# baseline (speedup 1.0000x reference)
"""Trainium2 Bass kernel for nn_MultiHeadAttention (B=4, S=2048, D=1024, H=16).

Sharding: 8 cores = 4 batches x 2 row-halves. Core (b, h) receives ONLY its
1024 rows of batch b (these serve as both its queries and its key-half). It
computes Q projections for its rows and K/V projections for its key-half;
the full K/V (all 2048 keys) is assembled with a pairwise AllGather
(replica groups [2b, 2b+1]; rank 0 holds rows 0-1023, so the gathered
chunks land in canonical key order on both cores). This halves the K/V
projection and transpose work versus computing full K/V per core.

Per-core dataflow (bf16 matmuls, fp32 PSUM accumulation):
  xh [1024,1024] --PE transpose--> xT [D, 1024]
  kT-own / v-own for the 1024 own keys (all 8 head pairs), staged to
  internal DRAM, AllGather, read back as kp [d-pair, 2048] and
  v_aug = [x @ Wv | ones] for all 16 key chunks.
  bk is dropped (its score offset is constant over keys -> cancels in
  softmax); bv contributes bv@Wo to y (sum(attn)=1) and is folded into bo
  on the host along with nothing else; bq is kept on the Q side.
  Per head pair p, per q-span of 512:
    scoresT[k,q] via row-paired (tile_position) K=64 matmuls
    exp on ACT (scale=1/8 folded in), table pre-warmed at kernel start
    attnV with M=65 aug (softmax denominator rides row 64 of PSUM)
    split epilogue: denominators and raw rows staged out of PSUM at once;
    reciprocal + K=1 broadcast matmuls + in-place normalize of oT deferred
    into the next span so the PE never waits on the DVE chain.
  y = oT^T @ Wo + bo'  (bo' = bo + bv@Wo precomputed on host)
  O-projection for q-span 0 interleaved into pair 7 span 1.
"""

import numpy as np
import ml_dtypes
from contextlib import ExitStack

import concourse.bass as bass
from concourse import bacc
import concourse.mybir as mybir
import concourse.tile as tile
from concourse.bass_utils import run_bass_kernel_spmd
from concourse.masks import make_identity

F32 = mybir.dt.float32
BF16 = mybir.dt.bfloat16
AF = mybir.ActivationFunctionType
NPBF16 = ml_dtypes.bfloat16

P = 128

N_CORES = 8
B_FULL, S_FULL, D_FULL = 4, 2048, 1024
H_FULL, DH = 16, 64
GROUPS = [[0, 1], [2, 3], [4, 5], [6, 7]]


def build_mha_nc(S=2048, Sq=1024, D=1024, H=16, scale=None):
    """Build the per-core Bass program. Returns nc."""
    assert D % P == 0 and S % P == 0 and Sq % P == 0 and H % 2 == 0
    ND = D // P            # d-tiles
    NS = S // P            # k-tiles over the full (gathered) key axis
    NSO = Sq // P          # own key chunks
    NPAIR = H // 2
    W65 = DH + 1           # augmented head width (v | ones)
    QSP = min(512, Sq)     # q span
    NQS = Sq // QSP
    KSP = 512              # span for own-kT projection (2 spans = 1024 keys)
    NKSO = Sq // KSP
    CSP = min(512, D)      # col span for v / out projections
    NCS = D // CSP
    HPS = CSP // DH        # heads per col-span in v projection
    if scale is None:
        scale = DH ** -0.5
    KV_K = NPAIR * P * Sq          # own kT elements
    KV_V = P * NSO * H * DH        # own v elements
    assert KV_K == KV_V
    KV_E = KV_K + KV_V

    nc = bacc.Bacc(target_bir_lowering=False, debug=False, num_devices=N_CORES)

    x = nc.dram_tensor("x", [Sq, D], BF16, kind="ExternalInput").ap()
    W = {n: nc.dram_tensor(n, [D, D], BF16, kind="ExternalInput").ap()
         for n in ("Wq", "Wk", "Wv", "Wo")}
    bias = {n: nc.dram_tensor(n, [D], F32, kind="ExternalInput").ap()
            for n in ("bq", "bo")}
    ones_d = nc.dram_tensor("cst_ones", [P, P], BF16, kind="ExternalInput").ap()
    y = nc.dram_tensor("y", [Sq, D], F32, kind="ExternalOutput").ap()
    kv_out = nc.dram_tensor("kv_out", [KV_E], BF16).ap()
    kv_all = nc.dram_tensor("kv_all", [2 * KV_E], BF16).ap()

    with tile.TileContext(nc) as tc, ExitStack() as top:
        top.enter_context(nc.allow_low_precision(
            reason="bf16 activations/weights with fp32 psum accumulation"))
        const = top.enter_context(tc.tile_pool(name="const", bufs=1))
        big = top.enter_context(tc.tile_pool(name="big", bufs=1))
        wp = top.enter_context(tc.tile_pool(name="wp", bufs=2))

        ident = const.tile([P, P], BF16)
        make_identity(nc, ident)
        # bf16 ones row: K=1 stationary broadcasting the softmax reciprocal
        ones_t = const.tile([1, DH], BF16)
        nc.vector.memset(ones_t, 1.0)
        # warm the ACT exp table while DMAs run
        warm = const.tile([1, 2], BF16)
        nc.scalar.activation(warm, ones_t[:, 0:2], AF.Exp, scale=1.0)

        # per-partition bias layouts: b_sb[p, j] = b[j*128 + p]
        bq_sb = const.tile([P, ND], F32)
        nc.gpsimd.dma_start(out=bq_sb, in_=bias["bq"].rearrange("(j p) -> p j", p=P))
        # bo broadcast across partitions (0-stride DRAM read)
        bo_bc = const.tile([P, D], F32)
        nc.gpsimd.dma_start(
            out=bo_bc,
            in_=bias["bo"].unsqueeze(0).partition_broadcast(P).squeeze(1),
        )

        oT = big.tile([P, ND, Sq], BF16)
        xT = big.tile([P, ND, Sq], BF16)
        qTs = big.tile([P, ND, Sq], BF16)
        v_sb = big.tile([P, NS, H * W65], BF16)
        v3 = v_sb.rearrange("p i (h w) -> p i h w", w=W65)
        kp = big.tile([P, NPAIR, S], BF16)

        # weight staging (wp "w" rotates: Wk, Wq, then Wo; Wv pinned)
        Wk_sb = wp.tile([P, ND, D], BF16, tag="w", name="Wk")
        Wq_sb = wp.tile([P, ND, D], BF16, tag="w", name="Wq")
        Wv_sb = wp.tile([P, ND, D], BF16, tag="wv", bufs=1)

        wo_box = {}

        # ---- prologue: own-half transposes + K/V projections + AllGather.
        # Its PSUM pools close before the attention pools open. ----
        with tc.tile_pool(name="xchunk", bufs=1) as xpool, \
             tc.tile_pool(name="tps", bufs=2, space="PSUM") as tpsum, \
             tc.tile_pool(name="ppE", bufs=3, space="PSUM") as ppE:
            xc = xpool.tile([P, NSO, D], BF16)
            for i in range(NSO):
                nc.sync.dma_start(out=xc[:, i, :], in_=x[i * P:(i + 1) * P, :])
            nc.sync.dma_start(
                out=Wk_sb, in_=W["Wk"].rearrange("(j p) c -> p j c", p=P))
            nc.sync.dma_start(
                out=Wv_sb, in_=W["Wv"].rearrange("(j p) c -> p j c", p=P))
            nc.sync.dma_start(
                out=Wq_sb, in_=W["Wq"].rearrange("(j p) c -> p j c", p=P))
            for i in range(NS):
                nc.sync.dma_start(out=v3[:, i, :, DH:DH + 1],
                                  in_=ones_d[:, 0:H].unsqueeze(2))
            for i in range(NSO):
                for j in range(ND):
                    tp = tpsum.tile([P, P], BF16, tag="tp")
                    nc.tensor.transpose(tp, xc[:, i, j * P:(j + 1) * P], ident)
                    nc.vector.tensor_copy(xT[:, j, i * P:(i + 1) * P], tp)
            # own-half kT for all pairs -> kp[:, p, 0:1024] (staging; the
            # gather readback rewrites the full key axis in canonical order)
            for p_ in range(NPAIR):
                for sp_ in range(NKSO):
                    ps = ppE.tile([P, KSP], F32, tag="pp", name=f"k_{p_}_{sp_}")
                    for j in range(ND):
                        nc.tensor.matmul(
                            ps,
                            Wk_sb[:, j, p_ * P:(p_ + 1) * P],
                            xT[:, j, sp_ * KSP:(sp_ + 1) * KSP],
                            start=(j == 0), stop=(j == ND - 1),
                        )
                    nc.vector.tensor_copy(
                        kp[:, p_, sp_ * KSP:(sp_ + 1) * KSP], ps)
            # own-half v -> v3[:, 0:8, :, :]
            for i in range(NSO):
                for sp_ in range(NCS):
                    ps = ppE.tile([P, CSP], F32, tag="pp", name=f"v_{i}_{sp_}")
                    for j in range(ND):
                        nc.tensor.matmul(
                            ps,
                            xT[:, j, i * P:(i + 1) * P],
                            Wv_sb[:, j, sp_ * CSP:(sp_ + 1) * CSP],
                            start=(j == 0), stop=(j == ND - 1),
                        )
                    nc.vector.tensor_copy(
                        v3[:, i, sp_ * HPS:(sp_ + 1) * HPS, 0:DH],
                        ps.rearrange("p (h w) -> p h w", w=DH),
                    )
            # stage own halves to DRAM, gather, read back both chunks
            nc.sync.dma_start(
                out=kv_out[0:KV_K].rearrange(
                    "(pr p k) -> p pr k", pr=NPAIR, p=P),
                in_=kp[:, :, 0:Sq],
            )
            nc.sync.dma_start(
                out=kv_out[KV_K:KV_E].rearrange(
                    "(p i h w) -> p i h w", p=P, i=NSO, h=H),
                in_=v3[:, 0:NSO, :, 0:DH],
            )
            nc.gpsimd.collective_compute(
                "AllGather",
                mybir.AluOpType.bypass,
                replica_groups=GROUPS,
                ins=[kv_out],
                outs=[kv_all],
            )
            kv6 = kv_all.rearrange("(c r e) -> c r e", c=2, r=2)
            for c in range(2):
                nc.sync.dma_start(
                    out=kp[:, :, c * Sq:(c + 1) * Sq],
                    in_=kv6[c, 0].rearrange("(pr p k) -> p pr k",
                                            pr=NPAIR, p=P),
                )
                nc.sync.dma_start(
                    out=v3[:, c * NSO:(c + 1) * NSO, :, 0:DH],
                    in_=kv6[c, 1].rearrange("(p i h w) -> p i h w",
                                            p=P, i=NSO, h=H),
                )

        with tc.tile_pool(name="exp", bufs=4) as exq, \
             tc.tile_pool(name="eps", bufs=2) as eps, \
             tc.tile_pool(name="scps", bufs=2, space="PSUM") as scps, \
             tc.tile_pool(name="ystg", bufs=2) as ystg, \
             tc.tile_pool(name="pps", bufs=2, space="PSUM") as pps, \
             tc.tile_pool(name="ops", bufs=2, space="PSUM") as opsum:

            def qT_proj(dc, sp):
                ps = pps.tile([P, QSP], F32, tag="pp", name=f"qps_{dc}_{sp}")
                for j in range(ND):
                    nc.tensor.matmul(
                        ps,
                        Wq_sb[:, j, dc * P:(dc + 1) * P],
                        xT[:, j, sp * QSP:(sp + 1) * QSP],
                        start=(j == 0), stop=(j == ND - 1),
                    )
                nc.vector.tensor_scalar_add(
                    qTs[:, dc, sp * QSP:(sp + 1) * QSP], ps,
                    bq_sb[:, dc:dc + 1])

            def load_wo():
                Wo_sb = wp.tile([P, ND, D], BF16, tag="w", name="Wo")
                nc.sync.dma_start(
                    out=Wo_sb, in_=W["Wo"].rearrange("(j p) c -> p j c", p=P))
                wo_box["Wo"] = Wo_sb

            def o_chunk(sc_i, spc):
                Wo_sb = wo_box["Wo"]
                ps = pps.tile([P, CSP], F32, tag="pp",
                              name=f"yps_{sc_i}_{spc}")
                for j in range(ND):
                    nc.tensor.matmul(
                        ps,
                        oT[:, j, sc_i * P:(sc_i + 1) * P],
                        Wo_sb[:, j, spc * CSP:(spc + 1) * CSP],
                        start=(j == 0), stop=(j == ND - 1),
                    )
                ysb = ystg.tile([P, CSP], F32, tag="ysb")
                nc.vector.tensor_add(
                    ysb, ps, bo_bc[:, spc * CSP:(spc + 1) * CSP])
                nc.sync.dma_start(
                    out=y[sc_i * P:(sc_i + 1) * P,
                          spc * CSP:(spc + 1) * CSP],
                    in_=ysb,
                )

            # queries for the first pairs; overlaps the gather latency
            for dc in (0, 1, 2):
                for sp_ in range(NQS):
                    qT_proj(dc, sp_)
            load_wo()

            # deferred-work schedule: (pair, span, kt) -> [thunks]
            jobs = {}

            def add(p, sp, kt, fn):
                jobs.setdefault((p, sp, kt), []).append(fn)

            for dc in range(3, ND):
                add(dc - 2, 0, 5, lambda dc=dc: qT_proj(dc, 0))
                add(dc - 2, 0, 10, lambda dc=dc: qT_proj(dc, 1))
            # O-projection span 0 interleaved into pair 7 span 1; slots
            # start after the deferred epilogue of (7, 0) fires at kt==2
            for (sc_i, spc), kt_ in zip(
                    [(si, c) for si in range(QSP // P) for c in range(NCS)],
                    (3, 4, 6, 8, 10, 12, 14, 15)):
                add(NPAIR - 1, 1, kt_, lambda a=sc_i, b=spc: o_chunk(a, b))

            # deferred epilogue part 2: reciprocal + broadcast + in-place
            # normalize of oT; runs inside the NEXT span's kt loop
            def epi_b(p, sp, den):
                nc.vector.reciprocal_approx_fast(den, den)
                rc16 = eps.tile([1, 2 * QSP], BF16, tag="rc16")
                nc.vector.tensor_copy(rc16, den)
                qsl = slice(sp * QSP, (sp + 1) * QSP)
                rb_ps = pps.tile([P, QSP], F32, tag="pp",
                                 name=f"rb_{p}_{sp}")
                nc.tensor.matmul(
                    rb_ps[0:DH, :], ones_t, rc16[:, 0:QSP],
                    start=True, stop=True,
                )
                nc.tensor.matmul(
                    rb_ps[DH:P, :], ones_t, rc16[:, QSP:2 * QSP],
                    start=True, stop=True,
                )
                rb = eps.tile([P, QSP], F32, tag="rb")
                nc.vector.tensor_copy(rb, rb_ps)
                nc.vector.tensor_mul(oT[:, p, qsl], oT[:, p, qsl], rb)

            pending = []

            # ---- attention: pair-outer, span-inner ----
            for p in range(NPAIR):
                for sp in range(NQS):
                    qsl = slice(sp * QSP, (sp + 1) * QSP)
                    o_even = opsum.tile([W65, QSP], F32, tag="op")
                    o_odd = opsum.tile([W65, QSP], F32, tag="op")
                    for kt in range(NS):
                        if kt == 2 and pending:
                            pending.pop()()
                        for fn in jobs.get((p, sp, kt), ()):
                            fn()
                        sc = scps.tile([P, 2 * QSP], F32, tag="sc")
                        nc.tensor.matmul(
                            sc[:, 0:QSP],
                            kp[0:DH, p, kt * P:(kt + 1) * P],
                            qTs[0:DH, p, qsl],
                            start=True, stop=True,
                        )
                        nc.tensor.matmul(
                            sc[:, QSP:2 * QSP],
                            kp[DH:P, p, kt * P:(kt + 1) * P],
                            qTs[DH:P, p, qsl],
                            start=True, stop=True,
                        )
                        ex = exq.tile([P, 2 * QSP], BF16, tag="ex")
                        nc.scalar.activation(ex, sc, AF.Exp,
                                             scale=float(scale))
                        nc.tensor.matmul(
                            o_even,
                            v3[:, kt, 2 * p, :],
                            ex[:, 0:QSP],
                            start=(kt == 0), stop=(kt == NS - 1),
                        )
                        nc.tensor.matmul(
                            o_odd,
                            v3[:, kt, 2 * p + 1, :],
                            ex[:, QSP:2 * QSP],
                            start=(kt == 0), stop=(kt == NS - 1),
                        )
                    # epilogue part 1: stage denominators and raw rows out
                    # of PSUM so the accumulators free quickly
                    den = eps.tile([1, 2 * QSP], F32, tag="den")
                    nc.vector.tensor_copy(den[:, 0:QSP], o_even[DH:W65, :])
                    nc.vector.tensor_copy(den[:, QSP:2 * QSP],
                                          o_odd[DH:W65, :])
                    nc.vector.tensor_copy(oT[0:DH, p, qsl], o_even[0:DH, :])
                    nc.vector.tensor_copy(oT[DH:P, p, qsl], o_odd[0:DH, :])
                    pending.append(
                        lambda p=p, sp=sp, den=den: epi_b(p, sp, den))
            # flush the last deferred epilogue, then tail O-projection
            while pending:
                pending.pop()()
            for sc_i in range(QSP // P, Sq // P):
                for spc in range(NCS):
                    o_chunk(sc_i, spc)

    nc.compile()
    return nc


_NC = None


def _get_nc():
    global _NC
    if _NC is None:
        _NC = build_mha_nc(S=S_FULL, Sq=S_FULL // 2, D=D_FULL, H=H_FULL)
    return _NC


def shard_inputs(inputs):
    x = np.asarray(inputs["x"], dtype=np.float32).astype(NPBF16)
    wnames = ("Wq", "Wk", "Wv", "Wo")
    shared = {n: np.ascontiguousarray(
        np.asarray(inputs[n], dtype=np.float32).astype(NPBF16)) for n in wnames}
    shared["bq"] = np.ascontiguousarray(np.asarray(inputs["bq"], dtype=np.float32))
    # bv contributes bv @ Wo to y (attention rows sum to 1); fold into bo
    bv = np.asarray(inputs["bv"], dtype=np.float32)
    Wo = np.asarray(inputs["Wo"], dtype=np.float32)
    bo = np.asarray(inputs["bo"], dtype=np.float32)
    shared["bo"] = np.ascontiguousarray(bo + bv @ Wo)
    shared["cst_ones"] = np.ones((P, P), dtype=NPBF16)
    half = S_FULL // 2
    maps = []
    for c in range(N_CORES):
        b, h = divmod(c, 2)
        m = dict(shared)
        m["x"] = np.ascontiguousarray(x[b, h * half:(h + 1) * half])
        maps.append(m)
    return maps


def run(inputs, trace=False):
    nc = _get_nc()
    maps = shard_inputs(inputs)
    res = run_bass_kernel_spmd(nc, maps, list(range(N_CORES)), trace=trace)
    half = S_FULL // 2
    y = np.empty((B_FULL, S_FULL, D_FULL), dtype=np.float32)
    for c in range(N_CORES):
        b, h = divmod(c, 2)
        y[b, h * half:(h + 1) * half] = res.results[c]["y"]
    return y, res


def kernel(**inputs):
    y, _ = run(inputs, trace=False)
    return y
